# revision 18
# baseline (speedup 1.0000x reference)
import sys
if '/opt/trn_rl_repo' not in sys.path:
    sys.path.insert(0, '/opt/trn_rl_repo')
import contextlib
import numpy as np
import ml_dtypes

import concourse.bass as bass
import concourse.tile as tile
from concourse import bacc, mybir

F32 = mybir.dt.float32
BF16 = mybir.dt.bfloat16
AF = mybir.ActivationFunctionType

# problem constants (hardcoded per contract)
B, C, H, W = 8, 64, 64, 64
G, KH, KW = 4, 3, 3
K = KH * KW
CG = C // G              # 16
COFF = C * K * 3         # 1728
COUT = 64
N_CORES = 8

# canvas geometry: row = orig y + 6 (y in -6..69 -> 76 rows), col = orig x + 4 (x in -4..67 -> 72)
CR, CW = 76, 72
CH_STRIDE = CR * CW      # 5472

UT = 1024                # u-tile = 16 output rows x 64
NT = H * W // UT         # 4
UTR = UT // W            # 16

PASSES = [(0, 1), (2, 3), (4, 5), (6, 7), (8, 8)]  # tap pairs (k0, k1), pass 4 duplicates tap 8
WLO, WHI = -3, 3         # hat window

KYT = [k // 3 - 1 for k in range(K)]
KXT = [k % 3 - 1 for k in range(K)]

CANV_SPAN = 23 * CW      # sampling canvas span per (pass, ut)
MOV_SPAN = 18 * CW       # conv moving span (rows 16t-1 .. 16t+16)

OUT_I8 = False           # int8 output truncates (not rounds) -> ~3.5% rel err, fails gate
I8_SCALE = 127.0         # |out| < 1 for this problem's deterministic inputs


def _tile_meta():
    # partition layout of the offset-conv output tiles: p = 2c + d, run-length
    # spans of constant input group (for matmul piece splitting)
    meta = []
    for dim in range(3):
        for p, (k0, k1) in enumerate(PASSES):
            npart = 128
            ocs = np.array([dim * 576 + c * 9 + kk
                            for c in range(64) for kk in (k0, k1)], np.int64)
            gin = ocs // 432
            runs = []
            s = 0
            for i in range(1, npart + 1):
                if i == npart or gin[i] != gin[s]:
                    runs.append((s, i, int(gin[s])))
                    s = i
            meta.append((dim, p, npart, ocs, runs))
    return meta


_TILE_META = _tile_meta()


def _prep_consts(weight, bias, weight_off, bias_off):
    # offset-conv stationary: [15 tiles][3 ky][48=(cg,kx), up to 128=(c,delta)]
    woff = weight_off.reshape(COFF, CG, KH, KW)
    wstat = np.zeros((15, 3, 48, 128), np.float32)
    boff_t = np.zeros((128, 15), np.float32)
    for (dim, p, npart, ocs, runs) in _TILE_META:
        ti = dim * 5 + p
        boff_t[:npart, ti] = bias_off[ocs]
        for ky in range(3):
            for kx in range(3):
                # row = cg*3 + kx  (cg outer, kx inner)
                wstat[ti, ky, kx::3, :npart] = woff[ocs, :, ky, kx].T
    # main-conv stationary, block-diagonal: [128=(c,delta), 5 passes x 64 oc]
    # pass 4 duplicates tap 8 on both delta slots; weight placed only on delta=0
    wmain = np.zeros((128, 5 * 64), np.float32)
    for p, (k0, k1) in enumerate(PASSES):
        for c in range(64):
            g, cg = c // 16, c % 16
            for d, kk in enumerate((k0, k1)):
                if p == 4 and d == 1:
                    continue
                wmain[2 * c + d, p * 64 + 16 * g:p * 64 + 16 * g + 16] = \
                    weight[16 * g:16 * g + 16, cg, kk // 3, kk % 3]

    sel32 = np.zeros((128, 32), np.float32)
    for pp in range(128):
        sel32[pp, pp % 32] = 1.0

    hatb = np.zeros((128, 8), np.float32)
    for i, dlt in enumerate(range(-3, 4)):
        hatb[:, i] = -float(dlt)
    hatb[:, 7] = 1.0

    return {
        'wstat': np.ascontiguousarray(
            wstat.reshape(45, 48, 128).transpose(1, 0, 2).reshape(48, 45 * 128)
        ).astype(ml_dtypes.bfloat16),
        'wmain': np.ascontiguousarray(wmain),
        'boff': np.ascontiguousarray(boff_t),
        'bmain': np.ascontiguousarray(
            bias.reshape(64, 1) * (I8_SCALE if OUT_I8 else 1.0)),
        'sel32': sel32,
        'hatb': hatb,
    }


def _build():
    nc = bacc.Bacc("TRN2", target_bir_lowering=False, debug=False, num_devices=N_CORES)
    inps_d = nc.dram_tensor("inps", [C * H * W], BF16, kind="ExternalInput").ap()
    wstat_d = nc.dram_tensor("wstat", [48, 45 * 128], BF16, kind="ExternalInput").ap()
    wmain_d = nc.dram_tensor("wmain", [128, 5 * 64], F32, kind="ExternalInput").ap()
    boff_d = nc.dram_tensor("boff", [128, 15], F32, kind="ExternalInput").ap()
    bmain_d = nc.dram_tensor("bmain", [64, 1], F32, kind="ExternalInput").ap()
    sel32_d = nc.dram_tensor("sel32", [128, 32], F32, kind="ExternalInput").ap()
    hatb_d = nc.dram_tensor("hatb", [128, 8], F32, kind="ExternalInput").ap()
    out_d = nc.dram_tensor("out", [COUT, H * W],
                           mybir.dt.int8 if OUT_I8 else BF16,
                           kind="ExternalOutput").ap()
    inps_h = inps_d.tensor

    with tile.TileContext(nc) as tc:
        with contextlib.ExitStack() as ctx:
            cpool = ctx.enter_context(tc.tile_pool(name="const", bufs=1))
            czp = ctx.enter_context(tc.tile_pool(name="czp", bufs=1))
            canvp = ctx.enter_context(tc.tile_pool(name="canv", bufs=2))
            cfp = ctx.enter_context(tc.tile_pool(name="cf", bufs=2))
            movp = ctx.enter_context(tc.tile_pool(name="mov", bufs=1))
            cop = ctx.enter_context(tc.tile_pool(name="convout", bufs=2))
            hatp = ctx.enter_context(tc.tile_pool(name="hats", bufs=1))
            hxp = ctx.enter_context(tc.tile_pool(name="hx", bufs=1))
            smp = ctx.enter_context(tc.tile_pool(name="smp", bufs=1))
            sp = ctx.enter_context(tc.tile_pool(name="stile", bufs=1))
            outp = ctx.enter_context(tc.tile_pool(name="outb", bufs=2))
            psp = ctx.enter_context(tc.tile_pool(name="ps", bufs=2, space="PSUM"))
            psm = ctx.enter_context(tc.tile_pool(name="psm", bufs=1, space="PSUM"))

            wstat_t = cpool.tile([48, 45 * 128], BF16)
            wmain_t = cpool.tile([128, 5 * 64], F32)
            boff_t = cpool.tile([128, 15], F32)
            bmain_t = cpool.tile([64, 1], F32)
            sel32_t = cpool.tile([128, 32], F32)
            hatb_t = cpool.tile([128, 8], F32)
            nc.sync.dma_start(hatb_t[:], hatb_d[:])
            nc.sync.dma_start(wstat_t[:], wstat_d[:])
            nc.sync.dma_start(wmain_t[:], wmain_d[:])
            nc.sync.dma_start(boff_t[:], boff_d[:])
            nc.sync.dma_start(bmain_t[:], bmain_d[:])
            nc.sync.dma_start(sel32_t[:], sel32_d[:])

            # ---- canvas staging: inps (bf16) -> SBUF canvas zt (bf16, zero borders)
            zt = czp.tile([C, CH_STRIDE], BF16, tag="zt")
            nc.vector.memset(zt[:], 0.0)
            nc.sync.dma_start(
                zt[:, 6 * CW + 4: 6 * CW + 4 + H * CW]
                .rearrange("a (r w) -> a r w", w=CW)[:, :, :W],
                bass.AP(inps_h, 0, [[H * W, C], [W, H], [1, W]]),
            )
            zt_h = zt[:].tensor
            zt_off = zt[:].offset
            zt_pw = zt[:].ap[0][0]  # partition stride (elements)

            for t in range(NT):
                # conv moving tiles per input group: [48=(cg,kx), 18 rows x 72]
                movs = []
                for gi in range(4):
                    mt = movp.tile([48, MOV_SPAN], BF16, tag=f"mov{gi}")
                    base = (16 * t + 5) * CW + 3   # rows 16t-1.., col base kx-1+4
                    nc.sync.dma_start(
                        mt[:],
                        bass.AP(zt_h, zt_off + 16 * gi * zt_pw + base,
                                [[zt_pw, 16], [1, 3], [1, MOV_SPAN]]),
                    )
                    movs.append(mt)

                s_tiles = []
                for p, (k0, k1) in enumerate(PASSES):
                    npart = 128
                    # --- offset conv: dy, dx, mask(raw->exp) tiles
                    couts = []
                    for dim in range(3):
                        ti = dim * 5 + p
                        runs = _TILE_META[ti][4]
                        co = cop.tile([npart, UT], F32, tag=f"co{dim}")
                        func = AF.Exp if dim == 2 else AF.Identity
                        # split runs into partition-quadrant-legal pieces
                        pieces = []
                        for (r0, r1, gi) in runs:
                            x = r0
                            while x < r1:
                                if x == 0:
                                    e = r1
                                elif x % 64 == 0:
                                    e = min(r1, x + 64)
                                else:
                                    e = min(r1, (x // 32 + 1) * 32)
                                pieces.append((x, e, gi))
                                x = e
                        for (r0, r1, gi) in pieces:
                            ps_t = psp.tile([r1 - r0, UT], F32, tag="convps")
                            for half in range(2):
                                for ky in range(3):
                                    mv = movs[gi][:, ky * CW + half * 8 * CW: ky * CW + half * 8 * CW + 8 * CW]
                                    mv = mv.rearrange("a (r w) -> a r w", w=CW)[:, :, :64]
                                    nc.tensor.matmul(
                                        ps_t[:, half * 512:(half + 1) * 512],
                                        wstat_t[:, (ti * 3 + ky) * 128 + r0:(ti * 3 + ky) * 128 + r1],
                                        mv,
                                        start=(ky == 0),
                                        stop=(ky == 2),
                                    )
                            nc.scalar.activation(co[r0:r1, :], ps_t[:], func,
                                                 bias=boff_t[r0:r1, ti:ti + 1], scale=1.0)
                        couts.append(co)
                    dy_t, dx_t, me_t = couts

                    # --- softmax normalization across groups (partition stride 32)
                    nsum = 32
                    sel_t = sel32_t
                    ms_ps = psm.tile([nsum, UT], F32, tag="mps")
                    for half in range(2):
                        nc.tensor.matmul(
                            ms_ps[:, half * 512:(half + 1) * 512],
                            sel_t[:npart, :nsum],
                            me_t[:, half * 512:(half + 1) * 512],
                            start=True, stop=True,
                        )
                    rec_t = smp.tile([nsum, UT], F32, tag="rec")
                    nc.vector.reciprocal(rec_t[:], ms_ps[:])
                    recb_t = smp.tile([npart, UT], F32, tag="recb")
                    for q in range(npart // nsum):
                        nc.sync.dma_start(recb_t[nsum * q:nsum * q + nsum, :], rec_t[:])
                    mask_t = smp.tile([npart, UT], F32, tag="mask")
                    nc.vector.tensor_mul(mask_t[:], me_t[:], recb_t[:])

                    # --- sampling canvas: partition (c, delta), pre-shifted by tap base
                    ctb = canvp.tile([npart, CANV_SPAN], BF16, tag="canvt")
                    cb0 = (16 * t + KYT[k0] + 3) * CW + KXT[k0]
                    cb1 = (16 * t + KYT[k1] + 3) * CW + KXT[k1]
                    nc.sync.dma_start(
                        ctb[:],
                        bass.AP(zt_h, zt_off + cb0,
                                [[zt_pw, 64], [cb1 - cb0, 2], [1, CANV_SPAN]]),
                    )
                    ct = cfp.tile([npart, CANV_SPAN], F32, tag="ctf")
                    nc.scalar.copy(ct[:], ctb[:])

                    # --- hat weights in x (kept), y (on the fly)
                    habs = hatp.tile([npart, UT], F32, tag="habs")
                    hx = []
                    for i, dlt in enumerate(range(WLO, WHI + 1)):
                        h = hxp.tile([npart, UT], F32, tag=f"hx{i}")
                        nc.scalar.activation(habs[:], dx_t[:], AF.Abs, bias=hatb_t[:npart, i:i + 1], scale=1.0)
                        nc.scalar.activation(h[:], habs[:], AF.Relu, bias=hatb_t[:npart, 7:8], scale=-1.0)
                        hx.append(h)

                    # --- 7x7 hat window accumulation
                    acc = smp.tile([npart, UT], F32, tag="acc")
                    tmp = smp.tile([npart, UT], F32, tag="tmp")
                    rowt = smp.tile([npart, UT], F32, tag="rowt")
                    tmp2 = smp.tile([npart, UT], F32, tag="tmp2")
                    rowt2 = smp.tile([npart, UT], F32, tag="rowt2")
                    rowtb = smp.tile([npart, UT], F32, tag="rowtb")
                    rowt2b = smp.tile([npart, UT], F32, tag="rowt2b")
                    hyc = hatp.tile([npart, UT], F32, tag="hyc")
                    for iy, dly in enumerate(range(WLO, WHI + 1)):
                        tmp_c = tmp
                        tmp2_c = tmp2
                        nc.scalar.activation(habs[:], dy_t[:], AF.Abs, bias=hatb_t[:npart, iy:iy + 1], scale=1.0)
                        nc.scalar.activation(hyc[:], habs[:], AF.Relu, bias=hatb_t[:npart, 7:8], scale=-1.0)
                        # x-window split: ix 0..3 on DVE (tmp), ix 4..6 on GPSIMD (tmp2)
                        for ix, dlx in enumerate(range(WLO, WHI + 1)):
                            off = (3 + dly) * CW + 4 + dlx
                            xap = ct[:, off:off + UTR * CW].rearrange("a (r w) -> a r w", w=CW)[:, :, :64]
                            if ix < 4:
                                eng, dtile, first = nc.vector, tmp_c, ix == 0
                                rtile = rowt if ix % 2 else rowtb
                            else:
                                eng, dtile, first = nc.gpsimd, tmp2_c, ix == 4
                                rtile = rowt2 if ix % 2 else rowt2b
                            dst = dtile if first else rtile
                            eng.tensor_mul(
                                dst[:].rearrange("a (r w) -> a r w", w=64),
                                hx[ix][:].rearrange("a (r w) -> a r w", w=64),
                                xap,
                            )
                            if not first:
                                eng.tensor_add(dtile[:], dtile[:], rtile[:])
                        nc.vector.tensor_add(tmp_c[:], tmp_c[:], tmp2_c[:])
                        if iy == 0:
                            nc.vector.tensor_mul(acc[:], tmp_c[:], hyc[:])
                        else:
                            nc.vector.tensor_mul(tmp_c[:], tmp_c[:], hyc[:])
                            nc.vector.tensor_add(acc[:], acc[:], tmp_c[:])
                    st = sp.tile([npart, UT], F32, tag=f"s{p}")
                    nc.vector.tensor_mul(st[:], acc[:], mask_t[:])
                    s_tiles.append(st)

                po = psm.tile([64, UT], F32, tag="mainps")
                for half in range(2):
                    for p in range(5):
                        nc.tensor.matmul(
                            po[:, half * 512:(half + 1) * 512],
                            wmain_t[:, p * 64:(p + 1) * 64],
                            s_tiles[p][:, half * 512:(half + 1) * 512],
                            start=(p == 0),
                            stop=(p == 4),
                        )
                ob = outp.tile([64, UT], mybir.dt.int8 if OUT_I8 else BF16, tag="ob")
                nc.scalar.activation(ob[:], po[:], AF.Identity, bias=bmain_t[:],
                                     scale=I8_SCALE if OUT_I8 else 1.0)
                nc.sync.dma_start(out_d[:, t * UT:(t + 1) * UT], ob[:])

    nc.compile()
    return nc


_ST = {}

_CONST_NAMES = ['wstat', 'wmain', 'boff', 'bmain', 'sel32', 'hatb']


def _get_state():
    if 'run' in _ST:
        return _ST
    import jax
    from jax.sharding import Mesh, PartitionSpec, NamedSharding
    try:
        from jax.shard_map import shard_map
    except Exception:
        from jax.experimental.shard_map import shard_map
    from concourse import bass2jax
    from concourse.bass2jax import _bass_exec_p, partition_id_tensor

    nc = _ST.get('nc')
    if nc is None:
        nc = _build()
    bass2jax.install_neuronx_cc_hook()

    partition_name = nc.partition_id_tensor.name if nc.partition_id_tensor else None
    in_names, out_names, out_avals, zero_shapes = [], [], [], []
    for alloc in nc.m.functions[0].allocations:
        if not isinstance(alloc, mybir.MemoryLocationSet):
            continue
        name = alloc.memorylocations[0].name
        if alloc.kind == "ExternalInput":
            if name != partition_name:
                in_names.append(name)
        elif alloc.kind == "ExternalOutput":
            shape = tuple(alloc.tensor_shape)
            dtype = mybir.dt.np(alloc.dtype)
            out_names.append(name)
            out_avals.append(jax.core.ShapedArray(shape, dtype))
            zero_shapes.append((shape, dtype))
    n_params = len(in_names)
    n_outs = len(out_avals)
    all_in_names = list(in_names) + list(out_names)
    if partition_name is not None:
        all_in_names.append(partition_name)

    def _body(*args):
        operands = list(args)
        if partition_name is not None:
            operands.append(partition_id_tensor())
        outs = _bass_exec_p.bind(
            *operands,
            out_avals=tuple(out_avals),
            in_names=tuple(all_in_names),
            out_names=tuple(out_names),
            lowering_input_output_aliases=(),
            sim_require_finite=True,
            sim_require_nnan=True,
            nc=nc,
        )
        return tuple(outs)

    devices = jax.devices()[:N_CORES]
    mesh = Mesh(np.asarray(devices), ("core",))
    sh = NamedSharding(mesh, PartitionSpec("core"))
    in_specs = (PartitionSpec("core"),) * (n_params + n_outs)
    out_specs = (PartitionSpec("core"),) * n_outs
    donate = tuple(range(n_params, n_params + n_outs))
    run = jax.jit(
        shard_map(_body, mesh=mesh, in_specs=in_specs, out_specs=out_specs,
                  check_rep=False),
        donate_argnums=donate,
        keep_unused=True,
    )

    def _stage_fn(*xs):
        return xs

    stage = jax.jit(_stage_fn, out_shardings=tuple(sh for _ in _CONST_NAMES))

    _ST.update(run=run, stage=stage, in_names=in_names, out_names=out_names,
               zero_shapes=zero_shapes, nc=nc, const_host=None, const_dev=None,
               prev_out=None)
    return _ST


def _run_once(st, inps_b):
    args = []
    for n in st['in_names']:
        if n == 'inps':
            args.append(inps_b)
        else:
            args.append(st['const_dev'][n])
    for (shape, dtype) in st['zero_shapes']:
        if st['prev_out'] is not None:
            args.append(st['prev_out'])
        else:
            args.append(np.zeros((N_CORES * shape[0], *shape[1:]), dtype))
    outs = st['run'](*args)
    out_arr = outs[0]
    res = np.asarray(out_arr)   # blocks until ready; single fetch roundtrip
    st['prev_out'] = out_arr
    return res


def _reset_backend():
    # device left unrecoverable (e.g. NRT_EXEC_UNIT_UNRECOVERABLE): tear the
    # PJRT client down and rebuild jit wrappers + device-resident state, which
    # re-handshakes with the axon terminal like a fresh process would
    import jax
    try:
        from jax._src import api as _jax_api
        _jax_api.clear_backends()
    except Exception:
        pass
    try:
        jax.clear_caches()
    except Exception:
        pass
    for k in ('run', 'stage'):
        _ST.pop(k, None)
    _ST['const_dev'] = None
    _ST['const_host'] = None
    _ST['prev_out'] = None
    _ST['warmed'] = False


def _ensure_consts(st, key):
    ch = st['const_host']
    if ch is None or not all(np.array_equal(a, b) for a, b in zip(ch, key)):
        consts = _prep_consts(*key)
        glob = [np.concatenate([consts[n]] * N_CORES, axis=0) for n in _CONST_NAMES]
        dev = st['stage'](*glob)
        st['const_dev'] = dict(zip(_CONST_NAMES, dev))
        st['const_host'] = tuple(a.copy() for a in key)


def kernel(**inputs) -> np.ndarray:
    import time as _time
    st = _get_state()

    key = tuple(np.asarray(inputs[k], np.float32) for k in
                ('weight', 'bias', 'weight_off', 'bias_off'))
    _ensure_consts(st, key)

    inps_b = np.ascontiguousarray(
        np.asarray(inputs['inps'], np.float32).reshape(B * C * H * W)
    ).astype(ml_dtypes.bfloat16)

    try:
        res = _run_once(st, inps_b)
    except Exception:
        st['prev_out'] = None
        _time.sleep(1)
        try:
            res = _run_once(st, inps_b)
        except Exception:
            _reset_backend()
            _time.sleep(2)
            st = _get_state()
            _ensure_consts(st, key)
            res = _run_once(st, inps_b)

    if not st.get('warmed'):
        # first-call-only warmup: settle PJRT dispatch/transfer paths so later
        # timed calls see steady state
        st['warmed'] = True
        for _ in range(4):
            res = _run_once(st, inps_b)

    res = res.reshape(B, COUT, H, W).astype(np.float32)
    if OUT_I8:
        res /= I8_SCALE
    return res
